# revision 12
# baseline (speedup 1.0000x reference)
"""DMRL model kernel for 8 Trainium2 NeuronCores (Bass/Tile).

Strategy: data-parallel over batch (2048 rows/core). Per core, everything is
kept feature-on-partition ([128, cols]): the text MLP runs as float32r
matmuls; user/item rows are gathered with indirect DMA and PE-transposed;
factor norms / dot products / attention projections are computed with
selector matmuls whose outputs are STACKED 16 row-tiles deep across PSUM
partitions (shifted-window APs), so the per-factor softmax/softplus chain
runs at full 128-partition width. The softmax over the batch axis needs a
global denominator: each core emits per-tile partial sums of exp(logits)
plus the numerators; the (tiny) cross-core combine happens on host.
"""
import sys

sys.path.insert(0, "/opt/trn_rl_repo")

import numpy as np

import concourse.bass as bass
import concourse.mybir as mybir
import concourse.tile as tile
from concourse.masks import make_identity

# ---------------- problem constants (hardcoded) ----------------
B = 16384
N = 5            # 1 pos + 4 neg items
D = 128          # embedding dim
F = 4            # factors
FS = 32          # factor size
TD = 384         # text dim
HID = 150        # text MLP hidden
M = 2            # modalities
NCORES = 8
BL = B // NCORES        # 2048 rows per core
TW = 512                # rows-tile width
NT = BL * N // TW       # 20 rows-tiles per core
NTILE128 = BL * N // 128  # 80 gather tiles per core
f32 = mybir.dt.float32
f32r = mybir.dt.float32r
i32 = mybir.dt.int32
AF = mybir.ActivationFunctionType

_CACHE = {}


# ---------------- walrus workarounds ----------------
def _patch_tile():
    """This env's walrus rejects >1 sem wait per instruction; spread extras
    onto engine-local NoOps. Also split the Tile end-of-kernel drain waits."""
    if getattr(tile.TileContext, "_dmrl_patched", False):
        return

    def _split_waits(nc, limit=1):
        for fn in nc.m.functions:
            for blk in fn.blocks:
                insts = blk.instructions
                i = 0
                while i < len(insts):
                    inst = insts[i]
                    si = getattr(inst, "sync_info", None)
                    if si is not None and si.on_wait and len(si.on_wait) > limit:
                        waits = list(si.on_wait)
                        inst.sync_info = mybir.SyncInfo(
                            on_wait=waits[:limit],
                            on_update=list(si.on_update) if si.on_update else [],
                        )
                        pos = i
                        for j in range(limit, len(waits), limit):
                            nop = mybir.InstNoOp(
                                name=nc.get_next_instruction_name(), ins=[], outs=[]
                            )
                            nop.engine = inst.engine
                            nop.sync_info = mybir.SyncInfo(
                                on_wait=waits[j:j + limit], on_update=[]
                            )
                            nc.register_instruction(nop)
                            insts.insert(pos, nop)
                            pos += 1
                            i += 1
                    i += 1

    def _drain_and_barrier(self, tick_clock, wait_clock):
        nc = self.nc
        drain_inst = nc.sync.drain()
        wait_clock.add_sem_waits(
            drain_inst.ins, tile.ScopedClock({None: tick_clock.global_clock})
        )
        si = drain_inst.ins.sync_info
        waits = list(si.on_wait) if si is not None and si.on_wait else []
        if len(waits) > 1:
            drain_inst.ins.sync_info = mybir.SyncInfo(
                on_wait=waits[:1],
                on_update=list(si.on_update) if si.on_update else [],
            )
            for w in waits[1:]:
                nop = nc.sync.nop()
                nop.ins.sync_info = mybir.SyncInfo(on_wait=[w], on_update=[])
        nc.all_engine_barrier()
        assert self.sems is not None
        popped = nc._tile_sem_poison_stack.pop()
        assert popped is self._sem_poison
        nc.clear_and_free_semaphores(list(self.sems.allocated().values()))
        nc.all_engine_barrier()

    tile.TileContext._drain_and_barrier = _drain_and_barrier
    _orig_exit = tile.TileContext.__exit__

    def _exit(self, *a, **kw):
        r = _orig_exit(self, *a, **kw)
        _split_waits(self.nc, limit=1)
        return r

    tile.TileContext.__exit__ = _exit
    tile.TileContext._dmrl_patched = True


# ---------------- device kernel ----------------
def build_nc(reps=1):
    _patch_tile()
    nc = bass.Bass()

    # DRAM I/O (per core). Matmul-feeding tensors are declared float32r
    # (same bytes as f32; the PE rounds internally on read).
    text_t = nc.dram_tensor("text_t", [N, 3, 128, BL], f32r, kind="ExternalInput")
    u_idx_d = nc.dram_tensor("u_idx", [128, BL // 128], i32, kind="ExternalInput")
    it_idx_d = nc.dram_tensor("it_idx", [128, NTILE128], i32, kind="ExternalInput")
    utab_d = nc.dram_tensor("utab", [100000, D], f32, kind="ExternalInput")
    itab_d = nc.dram_tensor("itab", [100000, D], f32, kind="ExternalInput")
    w1t_d = nc.dram_tensor("w1t", [3, 128, HID], f32r, kind="ExternalInput")
    w2t_d = nc.dram_tensor("w2t", [HID, 128], f32r, kind="ExternalInput")
    b1_d = nc.dram_tensor("b1c", [HID, 1], f32, kind="ExternalInput")
    b2_d = nc.dram_tensor("b2c", [128, 1], f32, kind="ExternalInput")
    wbg_d = nc.dram_tensor("wbg", [3, 128, 248], f32r, kind="ExternalInput")
    srep_d = nc.dram_tensor("srepg", [128, 248], f32r, kind="ExternalInput")
    sui_d = nc.dram_tensor("suig", [128, 248], f32r, kind="ExternalInput")
    sut_d = nc.dram_tensor("sutg", [128, 248], f32r, kind="ExternalInput")
    w2bs_d = nc.dram_tensor("w2bs", [128, 128], f32r, kind="ExternalInput")
    ab1x2_d = nc.dram_tensor("ab1x2", [128, 1], f32, kind="ExternalInput")
    cvec_d = nc.dram_tensor("cvecs", [128, 1], f32, kind="ExternalInput")

    t_out_d = nc.dram_tensor("t_out", [2, 128, TW], f32, kind="ExternalOutput")
    s_out_d = nc.dram_tensor("s_out", [128, 2], f32, kind="ExternalOutput")

    with tile.TileContext(nc) as tc:
        with (
            tc.tile_pool(name="const", bufs=1) as cpool,
            tc.tile_pool(name="ubuf", bufs=1) as upool,
            tc.tile_pool(name="text", bufs=2) as tpool,
            tc.tile_pool(name="ie", bufs=2) as iepool,
            tc.tile_pool(name="work", bufs=2) as wpool,
            tc.tile_pool(name="chain", bufs=2) as chpool,
            tc.tile_pool(name="outb", bufs=1) as opool,
            tc.tile_pool(name="pmm", bufs=2, space="PSUM") as pmm,        # h1a (2 banks)
            tc.tile_pool(name="pmm1", bufs=1, space="PSUM") as pmm1,     # h1b, te (2)
            tc.tile_pool(name="pacc", bufs=1, space="PSUM") as pacc,     # z, nrm2, dots (3)
            tc.tile_pool(name="ptr", bufs=1, space="PSUM") as ptr,       # transposes/logits (1)
        ):
            # ---- constants ----
            ident = cpool.tile([128, 128], f32)
            make_identity(nc, ident[:])
            w1t = cpool.tile([128, 3 * HID], f32r)
            for c in range(3):
                nc.sync.dma_start(w1t[:, c * HID:(c + 1) * HID], w1t_d[c])
            w2ta = cpool.tile([128, 128], f32r)
            nc.sync.dma_start(w2ta[:], w2t_d[0:128, :])
            w2tb = cpool.tile([22, 128], f32r)
            nc.sync.dma_start(w2tb[:], w2t_d[128:150, :])
            b1a = cpool.tile([128, 1], f32)
            nc.sync.dma_start(b1a[:], b1_d[0:128, :])
            b1b = cpool.tile([22, 1], f32)
            nc.sync.dma_start(b1b[:], b1_d[128:150, :])
            b2c = cpool.tile([128, 1], f32)
            nc.sync.dma_start(b2c[:], b2_d[:])
            wbg = cpool.tile([128, 3 * 248], f32r)
            for c in range(3):
                nc.sync.dma_start(wbg[:, c * 248:(c + 1) * 248], wbg_d[c])
            srepg = cpool.tile([128, 248], f32r)
            nc.sync.dma_start(srepg[:], srep_d[:])
            suig = cpool.tile([128, 248], f32r)
            nc.sync.dma_start(suig[:], sui_d[:])
            sutg = cpool.tile([128, 248], f32r)
            nc.sync.dma_start(sutg[:], sut_d[:])
            w2bs = cpool.tile([128, 128], f32r)
            nc.sync.dma_start(w2bs[:], w2bs_d[:])
            ab1x2 = cpool.tile([128, 1], f32)
            nc.sync.dma_start(ab1x2[:], ab1x2_d[:])
            cvecs = cpool.tile([128, 1], f32)
            nc.sync.dma_start(cvecs[:], cvec_d[:])
            u_idx = cpool.tile([128, BL // 128], i32)
            nc.sync.dma_start(u_idx[:], u_idx_d[:])
            it_idx = cpool.tile([128, NTILE128], i32)
            nc.sync.dma_start(it_idx[:], it_idx_d[:])

            s_sb = opool.tile([128, 2], f32)

            for _rep in range(reps):
                _body(nc, tc, locals())

    return nc


def _body(nc, tc, env):
    (cpool, upool, tpool, iepool, wpool, chpool, opool, pmm, pmm1, pacc, ptr,
     ident, w1t, w2ta, w2tb, b1a, b1b, b2c, wbg, srepg, suig, sutg, w2bs,
     ab1x2, cvecs, u_idx, it_idx, s_sb,
     utab_d, itab_d, text_t, t_out_d, s_out_d) = (
        env["cpool"], env["upool"], env["tpool"], env["iepool"], env["wpool"],
        env["chpool"], env["opool"], env["pmm"], env["pmm1"], env["pacc"],
        env["ptr"], env["ident"], env["w1t"], env["w2ta"], env["w2tb"],
        env["b1a"], env["b1b"], env["b2c"], env["wbg"], env["srepg"],
        env["suig"], env["sutg"], env["w2bs"], env["ab1x2"], env["cvecs"],
        env["u_idx"], env["it_idx"], env["s_sb"], env["utab_d"], env["itab_d"],
        env["text_t"], env["t_out_d"], env["s_out_d"])
    if True:
        if True:
            # ---- user rows: gather -> transpose -> u_T, sq_u ----
            u_nat = upool.tile([128, BL], f32)
            for g in range(BL // 128):
                nc.gpsimd.indirect_dma_start(
                    out=u_nat[:, g * 128:(g + 1) * 128],
                    out_offset=None,
                    in_=utab_d[:],
                    in_offset=bass.IndirectOffsetOnAxis(ap=u_idx[:, g:g + 1], axis=0),
                )
            u_T = upool.tile([128, BL], f32r)
            for q4 in range(BL // TW):
                tp = ptr.tile([128, TW], f32, tag="tr")
                for j in range(4):
                    g = q4 * 4 + j
                    nc.tensor.transpose(
                        tp[:, j * 128:(j + 1) * 128],
                        u_nat[:, g * 128:(g + 1) * 128],
                        ident[:],
                    )
                nc.vector.tensor_copy(u_T[:, q4 * TW:(q4 + 1) * TW], tp[:])
            sq_u = upool.tile([128, BL], f32r)
            for q4 in range(BL // TW):
                sl = slice(q4 * TW, (q4 + 1) * TW)
                nc.vector.tensor_mul(
                    sq_u[:, sl], u_T[:, sl].bitcast(f32), u_T[:, sl].bitcast(f32)
                )

            # ---- group psums (z / nrm2 / dots), allocated per 16-tile group ----
            group_ps = {}

            def get_group_psums(grp):
                if grp not in group_ps:
                    group_ps[grp] = (
                        pacc.tile([128, TW], f32, tag="z", name="zq"),
                        pacc.tile([128, TW], f32, tag="n2", name="n2q"),
                        pacc.tile([128, TW], f32, tag="dt", name="dq"),
                    )
                return group_ps[grp]

            def emit_chain(grp, pw):
                zq, n2q, dq = group_ps.pop(grp)
                lnn = chpool.tile([128, TW], f32, tag="lnn")
                nc.scalar.activation(lnn[0:pw, :], n2q[0:pw, :], AF.Ln)
                invn = chpool.tile([128, TW], f32, tag="invn")
                nc.scalar.activation(invn[0:pw, :], lnn[0:pw, :], AF.Exp, scale=-0.5)
                zs = chpool.tile([128, TW], f32, tag="zs")
                nc.vector.tensor_mul(zs[0:pw, :], zq[0:pw, :], invn[0:pw, :])
                t1 = chpool.tile([128, TW], f32, tag="t1")
                nc.scalar.activation(
                    t1[0:pw, :], zs[0:pw, :], AF.Exp, scale=2.0, bias=ab1x2[0:pw, :]
                )
                t2 = chpool.tile([128, TW], f32, tag="t2")
                nc.scalar.activation(t2[0:pw, :], t1[0:pw, :], AF.Ln, bias=1.0)
                t3 = chpool.tile([128, TW], f32r, tag="t3")
                nc.scalar.activation(t3[0:pw, :], t2[0:pw, :], AF.Exp, scale=-1.0)
                logp = ptr.tile([128, TW], f32, tag="tr")
                nc.tensor.matmul(
                    logp[0:pw, :], w2bs[0:pw, 0:pw], t3[0:pw, :],
                    start=True, stop=True,
                )
                e_t = chpool.tile([128, TW], f32, tag="e")
                nc.scalar.activation(
                    e_t[0:pw, :], logp[0:pw, :], AF.Exp, bias=cvecs[0:pw, :],
                    accum_out=s_sb[0:pw, grp:grp + 1],
                )
                ed = chpool.tile([128, TW], f32, tag="ed")
                nc.scalar.activation(ed[0:pw, :], dq[0:pw, :], AF.Exp)
                sp = chpool.tile([128, TW], f32, tag="sp")
                nc.scalar.activation(sp[0:pw, :], ed[0:pw, :], AF.Ln, bias=1.0)
                tst = chpool.tile([128, TW], f32, tag="tst")
                nc.vector.tensor_mul(tst[0:pw, :], e_t[0:pw, :], sp[0:pw, :])
                nc.sync.dma_start(t_out_d[grp, 0:pw, :], tst[0:pw, :])

            # ---- main loop over slabs ----
            for slab in range(N):
                # text (already feature-major in DRAM)
                tx = tpool.tile([128, 3 * BL], f32r, tag="tx")
                for c in range(3):
                    nc.sync.dma_start(
                        tx[:, c * BL:(c + 1) * BL], text_t[slab, c]
                    )
                # item rows for this slab: gather 16x128, transpose
                ie_nat = iepool.tile([128, BL], f32, tag="ienat")
                for g in range(BL // 128):
                    nc.gpsimd.indirect_dma_start(
                        out=ie_nat[:, g * 128:(g + 1) * 128],
                        out_offset=None,
                        in_=itab_d[:],
                        in_offset=bass.IndirectOffsetOnAxis(
                            ap=it_idx[:, slab * 16 + g:slab * 16 + g + 1], axis=0
                        ),
                    )
                ie_T = iepool.tile([128, BL], f32r, tag="ieT")
                for q4 in range(BL // TW):
                    tp = ptr.tile([128, TW], f32, tag="tr")
                    for j in range(4):
                        g = q4 * 4 + j
                        nc.tensor.transpose(
                            tp[:, j * 128:(j + 1) * 128],
                            ie_nat[:, g * 128:(g + 1) * 128],
                            ident[:],
                        )
                    nc.vector.tensor_copy(ie_T[:, q4 * TW:(q4 + 1) * TW], tp[:])

                for i in range(BL // TW):     # 4 rows-tiles per slab
                    tg = slab * 4 + i         # global tile id 0..19
                    grp = tg // 16
                    tl = tg % 16              # tile-in-group
                    win = (120 - 8 * tl, 248 - 8 * tl)
                    rs = slice(i * TW, (i + 1) * TW)
                    first = tl == 0           # first writer of the group psum
                    last = tl == 15 or tg == NT - 1

                    # --- text MLP ---
                    h1a_ps = pmm.tile([128, TW], f32, tag="h1a")
                    for c in range(3):
                        nc.tensor.matmul(
                            h1a_ps[:],
                            w1t[:, c * HID:c * HID + 128],
                            tx[:, c * BL + i * TW:c * BL + (i + 1) * TW],
                            start=(c == 0), stop=(c == 2),
                        )
                    h1b_ps = pmm1.tile([22, TW], f32, tag="h1b")
                    for c in range(3):
                        nc.tensor.matmul(
                            h1b_ps[:],
                            w1t[:, c * HID + 128:c * HID + 150],
                            tx[:, c * BL + i * TW:c * BL + (i + 1) * TW],
                            start=(c == 0), stop=(c == 2),
                        )
                    h1a = wpool.tile([128, TW], f32r, tag="h1as")
                    nc.scalar.activation(
                        h1a[:], h1a_ps[:], AF.Prelu, bias=b1a[:], alpha=0.01
                    )
                    h1b = wpool.tile([22, TW], f32r, tag="h1bs")
                    nc.scalar.activation(
                        h1b[:], h1b_ps[:], AF.Prelu, bias=b1b[:], alpha=0.01
                    )
                    te_ps = pmm1.tile([128, TW], f32, tag="te")
                    nc.tensor.matmul(te_ps[:], w2ta[:], h1a[:], start=True, stop=False)
                    nc.tensor.matmul(te_ps[:], w2tb[:], h1b[:], start=False, stop=True)
                    te = wpool.tile([128, TW], f32r, tag="te_s")
                    nc.scalar.activation(
                        te[:], te_ps[:], AF.Prelu, bias=b2c[:], alpha=0.01
                    )

                    # --- squares & products ---
                    sq_ie = wpool.tile([128, TW], f32r, tag="sqie")
                    nc.gpsimd.tensor_mul(
                        sq_ie[:], ie_T[:, rs].bitcast(f32), ie_T[:, rs].bitcast(f32)
                    )
                    sq_te = wpool.tile([128, TW], f32r, tag="sqte")
                    nc.vector.tensor_mul(
                        sq_te[:], te[:].bitcast(f32), te[:].bitcast(f32)
                    )
                    p_ui = wpool.tile([128, TW], f32r, tag="pui")
                    nc.vector.tensor_mul(
                        p_ui[:], u_T[:, rs].bitcast(f32), ie_T[:, rs].bitcast(f32)
                    )
                    p_ut = wpool.tile([128, TW], f32r, tag="put")
                    nc.vector.tensor_mul(
                        p_ut[:], u_T[:, rs].bitcast(f32), te[:].bitcast(f32)
                    )

                    # --- stacked attention matmuls ---
                    zq, n2q, dq = get_group_psums(grp)
                    for c, rhs in ((0, u_T[:, rs]), (1, ie_T[:, rs]), (2, te[:])):
                        nc.tensor.matmul(
                            zq[:], wbg[:, c * 248 + win[0]:c * 248 + win[1]], rhs,
                            start=(first and c == 0), stop=(last and c == 2),
                            skip_group_check=True,
                        )
                    for ci, rhs in ((0, sq_u[:, rs]), (1, sq_ie[:]), (2, sq_te[:])):
                        nc.tensor.matmul(
                            n2q[:], srepg[:, win[0]:win[1]], rhs,
                            start=(first and ci == 0), stop=(last and ci == 2),
                            skip_group_check=True,
                        )
                    nc.tensor.matmul(
                        dq[:], suig[:, win[0]:win[1]], p_ui[:],
                        start=first, stop=False, skip_group_check=True,
                    )
                    nc.tensor.matmul(
                        dq[:], sutg[:, win[0]:win[1]], p_ut[:],
                        start=False, stop=last, skip_group_check=True,
                    )

                if slab == 3:
                    emit_chain(0, 128)
            emit_chain(1, 32)
            nc.sync.dma_start(s_out_d[:], s_sb[:])

    return nc


# ---------------- host side ----------------
def _build_constants(w1, b1, w2, b2, aw1, ab1, aw2):
    w1t = np.ascontiguousarray(w1.T.reshape(3, 128, HID))
    w2t = np.ascontiguousarray(w2.T)                     # [150, 128]
    b1c = np.ascontiguousarray(b1.reshape(HID, 1))
    b2c = np.ascontiguousarray(b2.reshape(128, 1))

    Wb = np.zeros((3, 128, 8), np.float32)
    k = np.arange(128)
    f = k // 32
    j = k % 32
    for c in range(3):
        for m in range(M):
            Wb[c, k, 2 * f + m] = aw1[f, m, c * FS + j]
    wbg = np.zeros((3, 128, 248), np.float32)
    wbg[:, :, 120:128] = Wb

    srep = np.zeros((128, 8), np.float32)
    sui = np.zeros((128, 8), np.float32)
    sut = np.zeros((128, 8), np.float32)
    srep[k, 2 * f] = 1.0
    srep[k, 2 * f + 1] = 1.0
    sui[k, 2 * f] = 1.0
    sut[k, 2 * f + 1] = 1.0
    srepg = np.zeros((128, 248), np.float32)
    srepg[:, 120:128] = srep
    suig = np.zeros((128, 248), np.float32)
    suig[:, 120:128] = sui
    sutg = np.zeros((128, 248), np.float32)
    sutg[:, 120:128] = sut

    W2b = np.zeros((8, 8), np.float32)
    for ff in range(F):
        for mm in range(M):
            for oo in range(M):
                W2b[2 * ff + mm, 2 * ff + oo] = aw2[ff, oo, mm]
    w2bs = np.zeros((128, 128), np.float32)
    for t in range(16):
        w2bs[8 * t:8 * t + 8, 8 * t:8 * t + 8] = -2.0 * W2b
    cvec8 = np.zeros(8, np.float32)
    ab1v8 = np.zeros(8, np.float32)
    for ff in range(F):
        for oo in range(M):
            cvec8[2 * ff + oo] = aw2[ff, oo, :].sum()
        for mm in range(M):
            ab1v8[2 * ff + mm] = ab1[ff, mm]
    cvecs = np.tile(cvec8, 16).reshape(128, 1).astype(np.float32)
    ab1x2 = np.tile(2.0 * ab1v8, 16).reshape(128, 1).astype(np.float32)

    return dict(
        w1t=w1t, w2t=w2t, b1c=b1c, b2c=b2c, wbg=wbg, srepg=srepg,
        suig=suig, sutg=sutg, w2bs=w2bs, ab1x2=ab1x2, cvecs=cvecs,
    )


def _run_spmd(nc, in_maps):
    from concourse.bass_utils import run_bass_kernel_spmd

    return run_bass_kernel_spmd(nc, in_maps, list(range(NCORES)))


def kernel(**inputs):
    batch = np.asarray(inputs["batch"]).astype(np.int64)
    text = np.asarray(inputs["text"], dtype=np.float32)
    utab = np.ascontiguousarray(np.asarray(inputs["user_table"], dtype=np.float32))
    itab = np.ascontiguousarray(np.asarray(inputs["item_table"], dtype=np.float32))
    consts = _build_constants(
        np.asarray(inputs["w1"], np.float32), np.asarray(inputs["b1"], np.float32),
        np.asarray(inputs["w2"], np.float32), np.asarray(inputs["b2"], np.float32),
        np.asarray(inputs["aw1"], np.float32), np.asarray(inputs["ab1"], np.float32),
        np.asarray(inputs["aw2"], np.float32),
    )

    if "nc" not in _CACHE:
        _CACHE["nc"] = build_nc()
    nc = _CACHE["nc"]

    in_maps = []
    for c in range(NCORES):
        b0 = c * BL
        tsh = text[b0:b0 + BL]                                   # [BL, N, TD]
        text_tc = np.ascontiguousarray(
            tsh.transpose(1, 2, 0).reshape(N, 3, 128, BL)
        )
        users = batch[b0:b0 + BL, 0].astype(np.int32)
        items = batch[b0:b0 + BL, 1:].astype(np.int32)           # [BL, N]
        u_idx = np.ascontiguousarray(users.reshape(BL // 128, 128).T)
        it_idx = np.ascontiguousarray(
            items.T.reshape(N, BL // 128, 128).transpose(2, 0, 1).reshape(128, NTILE128)
        )
        in_maps.append(
            dict(
                text_t=text_tc, u_idx=u_idx, it_idx=it_idx,
                utab=utab, itab=itab, **consts,
            )
        )

    _CACHE["last_in_maps"] = in_maps
    res = _run_spmd(nc, in_maps)

    # ---- host combine ----
    # s_out[p, grp]: tile t = grp*16 + p//8 (grp1: t=16+p//8, p<32), q = p%8
    # t_out[grp, p, r]: value for (tile t, q, row r); slab n = t//4, b = (t%4)*TW + r
    S = np.zeros((N, 8), np.float64)
    for c in range(NCORES):
        s = res.results[c]["s_out"]
        sA = s[:, 0].reshape(16, 8)          # tiles 0..15
        sB = s[0:32, 1].reshape(4, 8)        # tiles 16..19
        st = np.concatenate([sA, sB], axis=0)            # [20, 8]
        S += st.reshape(N, 4, 8).sum(axis=1)
    ratings = np.zeros((B, N), np.float32)
    for c in range(NCORES):
        t_o = res.results[c]["t_out"]
        TA = t_o[0].reshape(16, 8, TW)
        TB = t_o[1][0:32].reshape(4, 8, TW)
        Tt = np.concatenate([TA, TB], axis=0)            # [20, 8, TW]
        Tt = Tt.reshape(N, 4, 8, TW)                     # [n, i, q, r]
        contrib = Tt / S[:, None, :, None]               # divide by global denom
        r_loc = contrib.sum(axis=2)                      # [n, i, r]
        ratings[c * BL:(c + 1) * BL, :] = (
            r_loc.transpose(1, 2, 0).reshape(BL, N)
        )
    return ratings
